# revision 22
# baseline (speedup 1.0000x reference)
"""Llama GQA attention (B=2,S=2048,H=32,KV=8,D=128,DM=4096) on 8 trn2 cores.

Sharding: DP=2 over sequences x TP=4 over heads. Core c = (b=c//4, g=c%4):
seq b's 2048 tokens, q-heads [8g,8g+8), kv-heads [2g,2g+2). Each core computes
its partial o-proj output; host sums the 4 TP partials per sequence.

Device layout: everything transposed ([feat, tok]) so the contraction dim is
always on partitions and no on-chip transposes are needed.
  qkv^T = W^T.T @ hidden^T          (W^T, hidden^T pre-transposed on host)
  S^T[j,i] = (k^T).T @ q^T          (contraction d=128 = one partition tile)
  P^T = exp(scale*S^T) * mask       (no max-subtraction: scores ~ N(0,1))
  C^T[d,i] = sum_j V[j,d].T ...     (lhsT=V tile, rhs=P^T, PSUM-accumulated)
  l[i] via DVE-accumulated P^T + one ones-row matmul per i-block;
  1/l via fast-approx DVE reciprocal + GpSimd partition broadcast.
  out^T = Wo^T.T @ (C^T / l)
RoPE rotate-half runs on DVE as two partition-shifted multiplies against a
sign-folded sin table (ssh), reading the bf16 copy of the projection PSUM.

Schedule: attention strip for i-block s (all heads) is emitted interleaved
with projection token-block s+1, so exp (ACT) and row-sum accumulation (DVE)
hide under the PE-bound projection matmuls. The final strip interleaves with
o-proj over token blocks 0..2; o-proj of token block 3 runs last.
"""

import numpy as np
import ml_dtypes

import concourse.bass as bass
import concourse.mybir as mybir
import concourse.tile as tile
from concourse.bass_utils import run_bass_kernel_spmd

F32 = mybir.dt.float32
BF16 = mybir.dt.bfloat16
BF = ml_dtypes.bfloat16


class Cfg:
    def __init__(self, S=2048, H=32, KV=8, D=128, TP=4, DP=2, TB=512, IB=512):
        self.S, self.H, self.KV, self.D = S, H, KV, D
        self.TP, self.DP = TP, DP
        self.DM = H * D
        self.HL = H // TP            # local q heads
        self.KVL = KV // TP          # local kv heads
        self.QF = self.HL * D        # local q feats
        self.KF = self.KVL * D
        self.VF = self.KVL * D
        self.LF = self.HL * D        # local o-proj contraction feats
        self.NKT = self.DM // 128    # K-tiles for qkv proj
        self.NQK = (self.QF + self.KF) // 128
        self.TB = min(TB, S)         # token block (qkv / o-proj moving dim)
        self.IB = min(IB, S)         # query block in attention
        self.ND = self.IB // 128     # j-tiles per i-block (diag patterns)
        self.GJ = 2 if self.ND >= 2 else 1   # j-tiles per exp group
        self.scale = float(D) ** -0.5


def build_kernel(tc, cfg):
    nc = tc.nc
    S, D = cfg.S, cfg.D
    TB, IB, ND, GJ = cfg.TB, cfg.IB, cfg.ND, cfg.GJ
    NKT, NQK = cfg.NKT, cfg.NQK
    NTB = S // TB
    NTT = TB // 128                  # tok tiles per block (for V)
    NIB = S // IB

    hid = nc.dram_tensor("hid_t", [cfg.DM, S], BF16, kind="ExternalInput").ap()
    wqk = nc.dram_tensor("wqk_t", [cfg.DM, cfg.QF + cfg.KF], BF16, kind="ExternalInput").ap()
    wv = nc.dram_tensor("wv_t", [cfg.DM, cfg.VF], BF16, kind="ExternalInput").ap()
    wo = nc.dram_tensor("wo_t", [cfg.LF, cfg.DM], BF16, kind="ExternalInput").ap()
    cos = nc.dram_tensor("cos_t", [128, S], F32, kind="ExternalInput").ap()
    ssh = nc.dram_tensor("ssh_t", [128, S], BF16, kind="ExternalInput").ap()
    msk = nc.dram_tensor("masks", [128, ND * IB], BF16, kind="ExternalInput").ap()
    out = nc.dram_tensor("out_t", [cfg.DM, S], F32, kind="ExternalOutput").ap()

    hid_r = hid.rearrange("(a p) t -> p a t", p=128)
    wqk_r = wqk.rearrange("(a p) f -> p a f", p=128)
    wv_r = wv.rearrange("(a p) f -> p a f", p=128)
    wo_r = wo.rearrange("(a p) f -> p a f", p=128)

    with tc.tile_pool(name="res", bufs=1) as res, \
         tc.tile_pool(name="p2", bufs=3) as p2, \
         tc.tile_pool(name="p2a", bufs=2) as p2a, \
         tc.tile_pool(name="p2r", bufs=2) as p2r, \
         tc.tile_pool(name="ps_s", bufs=2, space="PSUM") as ps_s, \
         tc.tile_pool(name="ps_lc", bufs=2, space="PSUM") as ps_lc:
        qkT = res.tile([128, NQK, S], BF16, tag="qkT")
        v_sb = res.tile([128, S // 128, cfg.VF], BF16, tag="v")
        attnT = res.tile([128, cfg.HL, S], BF16, tag="attnT")
        msk_t = res.tile([128, ND * IB], BF16, tag="msk")
        ones = res.tile([128, 1], BF16, tag="ones")
        wv_t = res.tile([128, NKT, cfg.VF], BF16, tag="wv")
        nc.vector.memset(ones[:], 1.0)

        def strip_head(s, h):
            """Attention for (head h, i-block s): full j-group pipeline."""
            ftk = cfg.HL + (h // (cfg.HL // cfg.KVL))
            hkv = h // (cfg.HL // cfg.KVL)
            isl = slice(s * IB, (s + 1) * IB)
            njt = ND * (s + 1)
            ngroups = njt // GJ
            cps = ps_lc.tile([128, IB], F32, tag="lc")
            pacc = p2a.tile([128, IB], BF16, tag="pacc")
            nc.vector.memset(pacc[:], 0.0)

            def toff(jj):
                """Trim offset: diagonal tile r only has valid i >= 128r."""
                r = jj - (njt - ND)
                return 128 * r if r > 0 else 0

            spss = [None] * ngroups
            for jg in range(ngroups + 1):
                if jg < ngroups:
                    sps = ps_s.tile([128, GJ, IB], F32, tag="sps")
                    spss[jg] = sps
                    for jl in range(GJ):
                        jj = jg * GJ + jl
                        o = toff(jj)
                        nc.tensor.matmul(
                            sps[:, jl, o:IB],
                            qkT[:, ftk, jj * 128:(jj + 1) * 128],
                            qkT[:, h, s * IB + o:(s + 1) * IB],
                            start=True, stop=True)
                if jg == 0:
                    continue
                g = jg - 1
                sps = spss[g]
                spss[g] = None
                pt = p2.tile([128, GJ, IB], BF16, tag="pt")
                nc.scalar.activation(
                    pt[:], sps[:],
                    mybir.ActivationFunctionType.Exp, scale=cfg.scale)
                for jl in range(GJ):
                    jj = g * GJ + jl
                    r = jj - (njt - ND)
                    o = toff(jj)
                    if r >= 0:  # diagonal tile: apply triangular mask
                        nc.vector.tensor_mul(
                            pt[:, jl, o:], pt[:, jl, o:],
                            msk_t[:, r * IB + o:(r + 1) * IB])
                    nc.vector.tensor_add(pacc[:, o:], pacc[:, o:], pt[:, jl, o:])
                for jl in range(GJ):
                    jj = g * GJ + jl
                    o = toff(jj)
                    nc.tensor.matmul(
                        cps[:, o:IB], v_sb[:, jj, hkv * D:(hkv + 1) * D],
                        pt[:, jl, o:IB],
                        start=(jj == 0), stop=(jj == njt - 1),
                        skip_group_check=True)
            lps = ps_lc.tile([1, IB], F32, tag="lc")
            nc.tensor.matmul(lps[0:1, :], ones[:], pacc[:], start=True, stop=True)
            rsb = p2r.tile([1, IB], F32, tag="rsb")
            nc.vector.reciprocal_approx_fast(rsb[:], lps[0:1, :])
            rb = p2r.tile([128, IB], F32, tag="rb")
            nc.gpsimd.partition_broadcast(rb[:], rsb[:])
            nc.vector.tensor_mul(attnT[:, h, isl], cps[:], rb[:])

        # ---- Projection token-blocks with interleaved attention strips ----
        with tc.tile_pool(name="p1", bufs=2) as p1, \
             tc.tile_pool(name="p1h", bufs=1) as p1h, \
             tc.tile_pool(name="p1w", bufs=2) as p1w, \
             tc.tile_pool(name="p1c", bufs=2) as p1c, \
             tc.tile_pool(name="ps_p", bufs=2, space="PSUM") as ps_p:
            # Warm the PE clock gate (HAM) during the startup DMA window so
            # the first real matmuls run at 2.4 GHz instead of 1.2 GHz.
            wrm = p1.tile([128, TB], BF16, tag="raw")
            nc.vector.memset(wrm[:], 0.0)
            wps = ps_p.tile([128, TB], F32, tag="ps")
            for _ in range(20):
                nc.tensor.matmul(wps[0:1, :], ones[:], wrm[:],
                                 start=True, stop=True)
            for tb in range(NTB):
                ts = slice(tb * TB, (tb + 1) * TB)
                hb = p1h.tile([128, NKT, TB], BF16, tag="hb")
                ck = NKT // 4
                hk = NKT // 2
                nc.sync.dma_start(hb[:, 0:ck, :], hid_r[:, 0:ck, ts])
                wt0 = None
                if tb == 0:   # ft0 weights right behind the first hb chunk
                    wt0 = p1w.tile([128, NKT, 128], BF16, tag="wt")
                    nc.sync.dma_start(wt0[:, 0:hk, :], wqk_r[:, 0:hk, 0:128])
                    nc.sync.dma_start(wt0[:, hk:, :], wqk_r[:, hk:, 0:128])
                for c in range(1, 4):   # chunked so first matmuls start early
                    nc.sync.dma_start(hb[:, c * ck:(c + 1) * ck, :],
                                      hid_r[:, c * ck:(c + 1) * ck, ts])
                cos_b = p1c.tile([128, TB], F32, tag="cos_b")
                nc.sync.dma_start(cos_b[:], cos[:, ts])
                ssh_b = p1c.tile([128, TB], BF16, tag="ssh_b")
                nc.sync.dma_start(ssh_b[:], ssh[:, ts])
                s = tb - 1   # strip hidden under this token block
                if s >= 0:
                    for h in range(3):
                        strip_head(s, h)
                for ft in range(NQK):
                    if ft == 0 and wt0 is not None:
                        wt = wt0
                    else:
                        wt = p1w.tile([128, NKT, 128], BF16, tag="wt")
                        nc.sync.dma_start(wt[:, 0:hk, :],
                                          wqk_r[:, 0:hk, ft * 128:(ft + 1) * 128])
                        nc.sync.dma_start(wt[:, hk:, :],
                                          wqk_r[:, hk:, ft * 128:(ft + 1) * 128])
                    if tb == 0 and ft < 4:   # stage wv in chunks + masks early
                        ck = NKT // 4
                        nc.sync.dma_start(wv_t[:, ft * ck:(ft + 1) * ck, :],
                                          wv_r[:, ft * ck:(ft + 1) * ck, :])
                        if ft == 0:
                            nc.sync.dma_start(msk_t[:], msk[:])
                    ps = ps_p.tile([128, TB], F32, tag="ps")
                    for kk in range(NKT):
                        nc.tensor.matmul(ps[:], wt[:, kk, :], hb[:, kk, :],
                                         start=(kk == 0), stop=(kk == NKT - 1))
                    # RoPE: bf16 copy, rotate-half on DVE (partition-shifted
                    # muls vs sign-folded sin), combine with cos
                    raw = p1.tile([128, TB], BF16, tag="raw")
                    nc.scalar.copy(raw[:], ps[:])
                    t2 = p1.tile([128, TB], F32, tag="t2")
                    nc.vector.tensor_mul(t2[0:64, :], raw[64:128, :], ssh_b[64:128, :])
                    nc.vector.tensor_mul(t2[64:128, :], raw[0:64, :], ssh_b[0:64, :])
                    t1 = p1.tile([128, TB], F32, tag="t1")
                    nc.vector.tensor_mul(t1[:], ps[:], cos_b[:])
                    nc.vector.tensor_add(qkT[:, ft, ts], t1[:], t2[:])
                    if s >= 0 and ft < 5:
                        strip_head(s, 3 + ft)
                for tt in range(NTT):
                    psv = ps_p.tile([128, cfg.VF], F32, tag="ps")
                    for kk in range(NKT):
                        nc.tensor.matmul(psv[:], hb[:, kk, tt * 128:(tt + 1) * 128],
                                         wv_t[:, kk, :],
                                         start=(kk == 0), stop=(kk == NKT - 1))
                    nc.vector.tensor_copy(v_sb[:, tb * NTT + tt, :], psv[:])

        # ---- o-proj: token blocks 0..NTB-2 first (interleaved with the final
        # attention strip), then the last token block reusing the resident
        # wo tiles (no weight reload) ----
        NOF = cfg.DM // 128
        NKF = cfg.LF // 128
        s_last = NIB - 1
        nheads = cfg.HL
        with tc.tile_pool(name="p3", bufs=2) as p3, \
             tc.tile_pool(name="p3w", bufs=NOF) as p3w, \
             tc.tile_pool(name="ps_o", bufs=2, space="PSUM") as ps_o:
            wts = []
            for of in range(NOF):
                wt = p3w.tile([128, NKF, 128], BF16, tag="wot")
                wts.append(wt)
                nc.sync.dma_start(wt[:], wo_r[:, :, of * 128:(of + 1) * 128])
                o_sb = p3.tile([128, (NTB - 1) * TB], F32, tag="o_sb")
                for tb in range(NTB - 1):
                    ts = slice(tb * TB, (tb + 1) * TB)
                    ps = ps_o.tile([128, TB], F32, tag="pso")
                    for kf in range(NKF):
                        nc.tensor.matmul(ps[:], wt[:, kf, :], attnT[:, kf, ts],
                                         start=(kf == 0), stop=(kf == NKF - 1))
                    nc.scalar.copy(o_sb[:, ts], ps[:])
                nc.sync.dma_start(out[of * 128:(of + 1) * 128, 0:(NTB - 1) * TB],
                                  o_sb[:])
                if of % 4 == 1 and of // 4 < nheads:
                    strip_head(s_last, of // 4)
            ts = slice((NTB - 1) * TB, NTB * TB)
            for of in range(NOF):
                o_sb = p3.tile([128, TB], F32, tag="o_sb2")
                ps = ps_o.tile([128, TB], F32, tag="pso")
                for kf in range(NKF):
                    nc.tensor.matmul(ps[:], wts[of][:, kf, :], attnT[:, kf, ts],
                                     start=(kf == 0), stop=(kf == NKF - 1))
                nc.scalar.copy(o_sb[:], ps[:])
                nc.sync.dma_start(out[of * 128:(of + 1) * 128, ts], o_sb[:])


def shard_inputs(hidden_states, cos, sin, qkv_weight, o_weight, cfg):
    """Host-side shard + transpose + bf16 cast. Returns list of 8 in_maps."""
    S, D, HL, KVL = cfg.S, cfg.D, cfg.HL, cfg.KVL
    H, KV = cfg.H, cfg.KV
    # RoPE tables (identical for both sequences - positions restart)
    cos_t = np.ascontiguousarray(cos[:S].T).astype(np.float32)
    sin_t = np.ascontiguousarray(sin[:S].T).astype(np.float32)
    # sign-folded sin for the DVE rotate-half:
    #   t2[0:64]  = raw[64:128] * ssh[64:128]  (= -x2 * sin)
    #   t2[64:128]= raw[0:64]   * ssh[0:64]    (=  x1 * sin)
    half = D // 2
    ssh_t = np.concatenate([sin_t[half:], -sin_t[:half]], 0).astype(BF)
    # causal diag masks: [128, ND*IB]
    IB, ND = cfg.IB, cfg.ND
    j = np.arange(128)[:, None]
    i = np.arange(IB)[None, :]
    masks = np.concatenate([(i >= 128 * r + j) for r in range(ND)], axis=1)
    masks = masks.astype(BF)

    in_maps = []
    for core in range(8):
        b, g = core // cfg.TP, core % cfg.TP
        tok = slice(b * S, (b + 1) * S)
        qr = slice(g * HL * D, (g + 1) * HL * D)
        kr = slice(H * D + g * KVL * D, H * D + (g + 1) * KVL * D)
        vr = slice((H + KV) * D + g * KVL * D, (H + KV) * D + (g + 1) * KVL * D)
        wqk_t = np.ascontiguousarray(
            np.concatenate([qkv_weight[qr], qkv_weight[kr]], 0).T).astype(BF)
        wv_t = np.ascontiguousarray(qkv_weight[vr].T).astype(BF)
        wo_t = np.ascontiguousarray(o_weight[:, qr].T).astype(BF)
        hid_t = np.ascontiguousarray(hidden_states[tok].T).astype(BF)
        in_maps.append({
            "hid_t": hid_t, "wqk_t": wqk_t, "wv_t": wv_t, "wo_t": wo_t,
            "cos_t": cos_t, "ssh_t": ssh_t, "masks": masks,
        })
    return in_maps


def unshard(results, cfg):
    T = cfg.DP * cfg.S
    out = np.zeros((T, cfg.DM), np.float32)
    for core, r in enumerate(results):
        b = core // cfg.TP
        out[b * cfg.S:(b + 1) * cfg.S] += r["out_t"].T
    return out.reshape(1, T, cfg.DM)


def _run(inputs, cfg, trace=False):
    import concourse.bacc as bacc
    nc = bacc.Bacc("TRN2", target_bir_lowering=False, debug=False,
                   enable_asserts=False, num_devices=8)
    with tile.TileContext(nc) as tc:
        build_kernel(tc, cfg)
    nc.compile()
    in_maps = shard_inputs(**inputs, cfg=cfg)
    res = run_bass_kernel_spmd(nc, in_maps, core_ids=list(range(8)), trace=trace)
    return unshard(res.results, cfg), res


def kernel(**inputs):
    out, _ = _run(inputs, Cfg())
    return out


# revision 23
# speedup vs baseline: 1.1780x; 1.1780x over previous
"""Llama GQA attention (B=2,S=2048,H=32,KV=8,D=128,DM=4096) on 8 trn2 cores.

Sharding: DP=2 over sequences x TP=4 over heads. Core c = (b=c//4, g=c%4):
seq b's 2048 tokens, q-heads [8g,8g+8), kv-heads [2g,2g+2). Each core computes
its partial o-proj output; host sums the 4 TP partials per sequence.

Device layout: everything transposed ([feat, tok]) so the contraction dim is
always on partitions and no on-chip transposes are needed.
  qkv^T = W^T.T @ hidden^T          (W^T, hidden^T pre-transposed on host)
  S^T[j,i] = (k^T).T @ q^T          (contraction d=128 = one partition tile)
  P^T = exp(scale*S^T) * mask       (no max-subtraction: scores ~ N(0,1))
  C^T[d,i] = sum_j V[j,d].T ...     (lhsT=V tile, rhs=P^T, PSUM-accumulated)
  l[i] via DVE-accumulated P^T + one ones-row matmul per i-block;
  1/l via fast-approx DVE reciprocal + GpSimd partition broadcast.
  out^T = Wo^T.T @ (C^T / l)
RoPE rotate-half runs on DVE as two partition-shifted multiplies against a
sign-folded sin table (ssh), reading the bf16 copy of the projection PSUM.

Schedule: attention strip for i-block s (all heads) is emitted interleaved
with projection token-block s+1, so exp (ACT) and row-sum accumulation (DVE)
hide under the PE-bound projection matmuls. The final strip interleaves with
o-proj over token blocks 0..2; o-proj of token block 3 runs last.
"""

import numpy as np
import ml_dtypes

import concourse.bass as bass
import concourse.mybir as mybir
import concourse.tile as tile
from concourse.bass_utils import run_bass_kernel_spmd

F32 = mybir.dt.float32
BF16 = mybir.dt.bfloat16
BF = ml_dtypes.bfloat16


class Cfg:
    def __init__(self, S=2048, H=32, KV=8, D=128, TP=4, DP=2, TB=512, IB=512):
        self.S, self.H, self.KV, self.D = S, H, KV, D
        self.TP, self.DP = TP, DP
        self.DM = H * D
        self.HL = H // TP            # local q heads
        self.KVL = KV // TP          # local kv heads
        self.QF = self.HL * D        # local q feats
        self.KF = self.KVL * D
        self.VF = self.KVL * D
        self.LF = self.HL * D        # local o-proj contraction feats
        self.NKT = self.DM // 128    # K-tiles for qkv proj
        self.NQK = (self.QF + self.KF) // 128
        self.TB = min(TB, S)         # token block (qkv / o-proj moving dim)
        self.IB = min(IB, S)         # query block in attention
        self.ND = self.IB // 128     # j-tiles per i-block (diag patterns)
        self.GJ = 2 if self.ND >= 2 else 1   # j-tiles per exp group
        self.scale = float(D) ** -0.5


def build_kernel(tc, cfg):
    nc = tc.nc
    S, D = cfg.S, cfg.D
    TB, IB, ND, GJ = cfg.TB, cfg.IB, cfg.ND, cfg.GJ
    NKT, NQK = cfg.NKT, cfg.NQK
    NTB = S // TB
    NTT = TB // 128                  # tok tiles per block (for V)
    NIB = S // IB

    hid = nc.dram_tensor("hid_t", [cfg.DM, S], BF16, kind="ExternalInput").ap()
    wqk = nc.dram_tensor("wqk_t", [cfg.DM, cfg.QF + cfg.KF], BF16, kind="ExternalInput").ap()
    wv = nc.dram_tensor("wv_t", [cfg.DM, cfg.VF], BF16, kind="ExternalInput").ap()
    wo = nc.dram_tensor("wo_t", [cfg.LF, cfg.DM], BF16, kind="ExternalInput").ap()
    cos = nc.dram_tensor("cos_t", [128, S], F32, kind="ExternalInput").ap()
    ssh = nc.dram_tensor("ssh_t", [128, S], BF16, kind="ExternalInput").ap()
    msk = nc.dram_tensor("masks", [128, ND * IB], BF16, kind="ExternalInput").ap()
    out = nc.dram_tensor("out_t", [cfg.DM, S], F32, kind="ExternalOutput").ap()

    hid_r = hid.rearrange("(a p) t -> p a t", p=128)
    wqk_r = wqk.rearrange("(a p) f -> p a f", p=128)
    wv_r = wv.rearrange("(a p) f -> p a f", p=128)
    wo_r = wo.rearrange("(a p) f -> p a f", p=128)

    with tc.tile_pool(name="res", bufs=1) as res, \
         tc.tile_pool(name="p2", bufs=3) as p2, \
         tc.tile_pool(name="p2a", bufs=2) as p2a, \
         tc.tile_pool(name="p2r", bufs=2) as p2r, \
         tc.tile_pool(name="ps_s", bufs=2, space="PSUM") as ps_s, \
         tc.tile_pool(name="ps_lc", bufs=2, space="PSUM") as ps_lc:
        qkT = res.tile([128, NQK, S], BF16, tag="qkT")
        v_sb = res.tile([128, S // 128, cfg.VF], BF16, tag="v")
        attnT = res.tile([128, cfg.HL, S], BF16, tag="attnT")
        msk_t = res.tile([128, ND * IB], BF16, tag="msk")
        ones = res.tile([128, 1], BF16, tag="ones")
        wv_t = res.tile([128, NKT, cfg.VF], BF16, tag="wv")
        nc.vector.memset(ones[:], 1.0)

        def strip_head(s, h):
            """Attention for (head h, i-block s): full j-group pipeline."""
            ftk = cfg.HL + (h // (cfg.HL // cfg.KVL))
            hkv = h // (cfg.HL // cfg.KVL)
            isl = slice(s * IB, (s + 1) * IB)
            njt = ND * (s + 1)
            ngroups = njt // GJ
            cps = ps_lc.tile([128, IB], F32, tag="lc")
            pacc = p2a.tile([128, IB], BF16, tag="pacc")
            nc.vector.memset(pacc[:], 0.0)

            def toff(jj):
                """Trim offset: diagonal tile r only has valid i >= 128r."""
                r = jj - (njt - ND)
                return 128 * r if r > 0 else 0

            spss = [None] * ngroups
            for jg in range(ngroups + 1):
                if jg < ngroups:
                    sps = ps_s.tile([128, GJ, IB], F32, tag="sps")
                    spss[jg] = sps
                    for jl in range(GJ):
                        jj = jg * GJ + jl
                        o = toff(jj)
                        nc.tensor.matmul(
                            sps[:, jl, o:IB],
                            qkT[:, ftk, jj * 128:(jj + 1) * 128],
                            qkT[:, h, s * IB + o:(s + 1) * IB],
                            start=True, stop=True)
                if jg == 0:
                    continue
                g = jg - 1
                sps = spss[g]
                spss[g] = None
                pt = p2.tile([128, GJ, IB], BF16, tag="pt")
                nc.scalar.activation(
                    pt[:], sps[:],
                    mybir.ActivationFunctionType.Exp, scale=cfg.scale)
                for jl in range(GJ):
                    jj = g * GJ + jl
                    r = jj - (njt - ND)
                    o = toff(jj)
                    if r >= 0:  # diagonal tile: apply triangular mask
                        nc.vector.tensor_mul(
                            pt[:, jl, o:], pt[:, jl, o:],
                            msk_t[:, r * IB + o:(r + 1) * IB])
                    nc.vector.tensor_add(pacc[:, o:], pacc[:, o:], pt[:, jl, o:])
                for jl in range(GJ):
                    jj = g * GJ + jl
                    o = toff(jj)
                    nc.tensor.matmul(
                        cps[:, o:IB], v_sb[:, jj, hkv * D:(hkv + 1) * D],
                        pt[:, jl, o:IB],
                        start=(jj == 0), stop=(jj == njt - 1),
                        skip_group_check=True)
            lps = ps_lc.tile([1, IB], F32, tag="lc")
            nc.tensor.matmul(lps[0:1, :], ones[:], pacc[:], start=True, stop=True)
            rsb = p2r.tile([1, IB], F32, tag="rsb")
            nc.vector.reciprocal_approx_fast(rsb[:], lps[0:1, :])
            rb = p2r.tile([128, IB], F32, tag="rb")
            nc.gpsimd.partition_broadcast(rb[:], rsb[:])
            nc.vector.tensor_mul(attnT[:, h, isl], cps[:], rb[:])

        # ---- Projection token-blocks with interleaved attention strips ----
        with tc.tile_pool(name="p1", bufs=2) as p1, \
             tc.tile_pool(name="p1h", bufs=1) as p1h, \
             tc.tile_pool(name="p1w", bufs=2) as p1w, \
             tc.tile_pool(name="p1c", bufs=2) as p1c, \
             tc.tile_pool(name="ps_p", bufs=2, space="PSUM") as ps_p:
            # Warm the PE clock gate (HAM) during the startup DMA window so
            # the first real matmuls run at 2.4 GHz instead of 1.2 GHz.
            wrm = p1.tile([128, TB], BF16, tag="raw")
            nc.vector.memset(wrm[:], 0.0)
            wps = ps_p.tile([128, TB], F32, tag="ps")
            for _ in range(28):
                nc.tensor.matmul(wps[0:1, :], ones[:], wrm[:],
                                 start=True, stop=True)
            for tb in range(NTB):
                ts = slice(tb * TB, (tb + 1) * TB)
                hb = p1h.tile([128, NKT, TB], BF16, tag="hb")
                ck = NKT // 4
                hk = NKT // 2
                nc.sync.dma_start(hb[:, 0:ck, :], hid_r[:, 0:ck, ts])
                wt0 = None
                if tb == 0:   # ft0 weights right behind the first hb chunk
                    wt0 = p1w.tile([128, NKT, 128], BF16, tag="wt")
                    nc.sync.dma_start(wt0[:, 0:hk, :], wqk_r[:, 0:hk, 0:128])
                    nc.sync.dma_start(wt0[:, hk:, :], wqk_r[:, hk:, 0:128])
                for c in range(1, 4):   # chunked so first matmuls start early
                    nc.sync.dma_start(hb[:, c * ck:(c + 1) * ck, :],
                                      hid_r[:, c * ck:(c + 1) * ck, ts])
                cos_b = p1c.tile([128, TB], F32, tag="cos_b")
                nc.sync.dma_start(cos_b[:], cos[:, ts])
                ssh_b = p1c.tile([128, TB], BF16, tag="ssh_b")
                nc.sync.dma_start(ssh_b[:], ssh[:, ts])
                s = tb - 1   # strip hidden under this token block
                if s >= 0:
                    for h in range(3):
                        strip_head(s, h)
                for ft in range(NQK):
                    if ft == 0 and wt0 is not None:
                        wt = wt0
                    else:
                        wt = p1w.tile([128, NKT, 128], BF16, tag="wt")
                        nc.sync.dma_start(wt[:, 0:hk, :],
                                          wqk_r[:, 0:hk, ft * 128:(ft + 1) * 128])
                        nc.sync.dma_start(wt[:, hk:, :],
                                          wqk_r[:, hk:, ft * 128:(ft + 1) * 128])
                    if tb == 0 and ft < 4:   # stage wv in chunks + masks early
                        ck = NKT // 4
                        nc.sync.dma_start(wv_t[:, ft * ck:(ft + 1) * ck, :],
                                          wv_r[:, ft * ck:(ft + 1) * ck, :])
                        if ft == 0:
                            nc.sync.dma_start(msk_t[:], msk[:])
                    ps = ps_p.tile([128, TB], F32, tag="ps")
                    for kk in range(NKT):
                        nc.tensor.matmul(ps[:], wt[:, kk, :], hb[:, kk, :],
                                         start=(kk == 0), stop=(kk == NKT - 1))
                    # RoPE: bf16 copy, rotate-half on DVE (partition-shifted
                    # muls vs sign-folded sin), combine with cos
                    raw = p1.tile([128, TB], BF16, tag="raw")
                    nc.scalar.copy(raw[:], ps[:])
                    t2 = p1.tile([128, TB], F32, tag="t2")
                    nc.vector.tensor_mul(t2[0:64, :], raw[64:128, :], ssh_b[64:128, :])
                    nc.vector.tensor_mul(t2[64:128, :], raw[0:64, :], ssh_b[0:64, :])
                    t1 = p1.tile([128, TB], F32, tag="t1")
                    nc.vector.tensor_mul(t1[:], ps[:], cos_b[:])
                    nc.vector.tensor_add(qkT[:, ft, ts], t1[:], t2[:])
                    if s >= 0 and ft < 5:
                        strip_head(s, 3 + ft)
                for tt in range(NTT):
                    psv = ps_p.tile([128, cfg.VF], F32, tag="ps")
                    for kk in range(NKT):
                        nc.tensor.matmul(psv[:], hb[:, kk, tt * 128:(tt + 1) * 128],
                                         wv_t[:, kk, :],
                                         start=(kk == 0), stop=(kk == NKT - 1))
                    nc.vector.tensor_copy(v_sb[:, tb * NTT + tt, :], psv[:])

        # ---- o-proj: token blocks 0..NTB-2 first (interleaved with the final
        # attention strip), then the last token block reusing the resident
        # wo tiles (no weight reload) ----
        NOF = cfg.DM // 128
        NKF = cfg.LF // 128
        s_last = NIB - 1
        nheads = cfg.HL
        with tc.tile_pool(name="p3", bufs=2) as p3, \
             tc.tile_pool(name="p3w", bufs=NOF) as p3w, \
             tc.tile_pool(name="ps_o", bufs=2, space="PSUM") as ps_o:
            wts = []
            for of in range(NOF):
                wt = p3w.tile([128, NKF, 128], BF16, tag="wot")
                wts.append(wt)
                nc.sync.dma_start(wt[:], wo_r[:, :, of * 128:(of + 1) * 128])
                o_sb = p3.tile([128, (NTB - 1) * TB], F32, tag="o_sb")
                for tb in range(NTB - 1):
                    ts = slice(tb * TB, (tb + 1) * TB)
                    ps = ps_o.tile([128, TB], F32, tag="pso")
                    for kf in range(NKF):
                        nc.tensor.matmul(ps[:], wt[:, kf, :], attnT[:, kf, ts],
                                         start=(kf == 0), stop=(kf == NKF - 1))
                    nc.scalar.copy(o_sb[:, ts], ps[:])
                nc.sync.dma_start(out[of * 128:(of + 1) * 128, 0:(NTB - 1) * TB],
                                  o_sb[:])
                if of % 4 == 1 and of // 4 < nheads:
                    strip_head(s_last, of // 4)
            ts = slice((NTB - 1) * TB, NTB * TB)
            for of in range(NOF):
                o_sb = p3.tile([128, TB], F32, tag="o_sb2")
                ps = ps_o.tile([128, TB], F32, tag="pso")
                for kf in range(NKF):
                    nc.tensor.matmul(ps[:], wts[of][:, kf, :], attnT[:, kf, ts],
                                     start=(kf == 0), stop=(kf == NKF - 1))
                nc.scalar.copy(o_sb[:], ps[:])
                nc.sync.dma_start(out[of * 128:(of + 1) * 128, ts], o_sb[:])


def shard_inputs(hidden_states, cos, sin, qkv_weight, o_weight, cfg):
    """Host-side shard + transpose + bf16 cast. Returns list of 8 in_maps."""
    S, D, HL, KVL = cfg.S, cfg.D, cfg.HL, cfg.KVL
    H, KV = cfg.H, cfg.KV
    # RoPE tables (identical for both sequences - positions restart)
    cos_t = np.ascontiguousarray(cos[:S].T).astype(np.float32)
    sin_t = np.ascontiguousarray(sin[:S].T).astype(np.float32)
    # sign-folded sin for the DVE rotate-half:
    #   t2[0:64]  = raw[64:128] * ssh[64:128]  (= -x2 * sin)
    #   t2[64:128]= raw[0:64]   * ssh[0:64]    (=  x1 * sin)
    half = D // 2
    ssh_t = np.concatenate([sin_t[half:], -sin_t[:half]], 0).astype(BF)
    # causal diag masks: [128, ND*IB]
    IB, ND = cfg.IB, cfg.ND
    j = np.arange(128)[:, None]
    i = np.arange(IB)[None, :]
    masks = np.concatenate([(i >= 128 * r + j) for r in range(ND)], axis=1)
    masks = masks.astype(BF)

    in_maps = []
    for core in range(8):
        b, g = core // cfg.TP, core % cfg.TP
        tok = slice(b * S, (b + 1) * S)
        qr = slice(g * HL * D, (g + 1) * HL * D)
        kr = slice(H * D + g * KVL * D, H * D + (g + 1) * KVL * D)
        vr = slice((H + KV) * D + g * KVL * D, (H + KV) * D + (g + 1) * KVL * D)
        wqk_t = np.ascontiguousarray(
            np.concatenate([qkv_weight[qr], qkv_weight[kr]], 0).T).astype(BF)
        wv_t = np.ascontiguousarray(qkv_weight[vr].T).astype(BF)
        wo_t = np.ascontiguousarray(o_weight[:, qr].T).astype(BF)
        hid_t = np.ascontiguousarray(hidden_states[tok].T).astype(BF)
        in_maps.append({
            "hid_t": hid_t, "wqk_t": wqk_t, "wv_t": wv_t, "wo_t": wo_t,
            "cos_t": cos_t, "ssh_t": ssh_t, "masks": masks,
        })
    return in_maps


def unshard(results, cfg):
    T = cfg.DP * cfg.S
    out = np.zeros((T, cfg.DM), np.float32)
    for core, r in enumerate(results):
        b = core // cfg.TP
        out[b * cfg.S:(b + 1) * cfg.S] += r["out_t"].T
    return out.reshape(1, T, cfg.DM)


def _run(inputs, cfg, trace=False):
    import concourse.bacc as bacc
    nc = bacc.Bacc("TRN2", target_bir_lowering=False, debug=False,
                   enable_asserts=False, num_devices=8)
    with tile.TileContext(nc) as tc:
        build_kernel(tc, cfg)
    nc.compile()
    in_maps = shard_inputs(**inputs, cfg=cfg)
    res = run_bass_kernel_spmd(nc, in_maps, core_ids=list(range(8)), trace=trace)
    return unshard(res.results, cfg), res


def kernel(**inputs):
    out, _ = _run(inputs, Cfg())
    return out
